# revision 65
# baseline (speedup 1.0000x reference)
"""Trainium2 Bass kernel for CAttention (contextual attention).

Math (per batch element, derived from the reference):
    x:    (c=128, h=64, w=64), flat (128, 4096); m: (1, 4096)
    k    = normalize_rows(x.reshape(c, hw).T + eps)          # (4096, 128)
    y    = 3x3 zero-padded box filter of x                   # (128, 4096)
      (the 3x3 sum-pool of the padded score map commutes with the 1x1-conv
       matmul and collapses onto x; border columns of the padded map only
       influence cropped-away outputs)
    S    = k @ y                                             # (4096 l, 4096 ij)
    att  = softmax over l (per column); constant-shift trick: softmax needs no
           per-column max because S is bounded (|S| <= ~34, col max >= ~11):
           u = exp(S - 20), att = u / colsum(u)
    rec  = k.T @ att                                         # (128, 4096)
    out  = rec * (1-m)/9 + x*m

Sharding: pure data parallel over batch (4) x output-column halves (2) = 8
cores, zero cross-core communication. Each core: full l = 4096, its 2048
output columns.

Per-core layout (l-orientation: l on partitions for scores):
    mm1:  scores[l_tile, ij] = xeps[:, l_tile].T @ y[:, ij]       (PE)
    exp:  u = Exp(scores * rscale[l] - 20)  per-partition scale    (ACT)
          (folds the k-row normalization into the softmax argument)
    mm2:  rec += kn[l_tile].T @ u   accumulated over 32 l-tiles    (PE)
    sums: sums += ones.T @ u        column sums, broadcast to 128p (PE)
    then R = 1/sums, out = rec*R*(1-m)/9 + x*m                     (DVE)
"""

import numpy as np

EPS = 1e-7
SHIFT = 20.0
C = 128          # channels
L = 4096         # spatial locations (l axis)
HALF = 2048      # output columns per core
BLK = 1024       # ij block (psum-bank sized: 2 banks)
NLT = 32         # l tiles of 128
YW = 2176        # xyh width: 34 padded image rows x 64
USE_F32R = True  # fast fp32 matmul mode on the PE (matmul operands stored as
                 # float32r; producers round to it, ~17-bit mantissa)

_CACHE = {}


def _build_program():
    import concourse.bass as bass
    import concourse.bacc as bacc
    import concourse.tile as tile
    import concourse.mybir as mybir

    F32 = mybir.dt.float32
    F32R = mybir.dt.float32r
    AF = mybir.ActivationFunctionType
    ALU = mybir.AluOpType

    nc = bacc.Bacc("TRN2", target_bir_lowering=False, num_swdge_queues=4)

    x_d = nc.dram_tensor("x", [C, L], F32, kind="ExternalInput")
    # xt pre-tiled on host to SBUF layout: xtt[p, t*128+c] = x[c, t*128+p]
    xt_d = nc.dram_tensor("xt", [C, L], F32, kind="ExternalInput")
    xyh_d = nc.dram_tensor("xyh", [C, YW], F32, kind="ExternalInput")
    mrep_d = nc.dram_tensor("mrep", [C, HALF], F32, kind="ExternalInput")
    out_d = nc.dram_tensor("out", [C, HALF], F32, kind="ExternalOutput")

    FMM = F32R if USE_F32R else F32  # dtype for matmul operand tiles

    def mc(ap):
        return ap

    with tile.TileContext(nc) as tc:
        with (
            tc.tile_pool(name="big", bufs=1) as big,
            tc.tile_pool(name="small", bufs=1) as small,
            tc.tile_pool(name="sqs", bufs=2) as sqs,
            tc.tile_pool(name="upool", bufs=6) as upool,
            tc.tile_pool(name="vpool", bufs=3) as vpool,
            tc.tile_pool(name="wpool", bufs=4) as wpool,
            tc.tile_pool(name="opool", bufs=3) as opool,
            tc.tile_pool(name="ps_sc", bufs=2, space=bass.MemorySpace.PSUM) as ps_sc,
            tc.tile_pool(name="ps_rec", bufs=1, space=bass.MemorySpace.PSUM) as ps_rec,
            tc.tile_pool(name="ps_sum", bufs=1, space=bass.MemorySpace.PSUM) as ps_sum,
        ):
            # ---- persistent SBUF tensors ----
            x_sb = big.tile([C, L], F32, tag="x_sb")
            xeps = big.tile([C, L], FMM, tag="xeps")
            xt_sb = big.tile([C, L], F32, tag="xt_sb")     # 32 tiles (128l, 128c)
            kn = big.tile([C, L], FMM, tag="kn")           # normalized k, l-major tiles
            xyh_sb = big.tile([C, YW], F32, tag="xyh_sb")
            y1 = big.tile([C, YW], F32, tag="y1")
            y_t = big.tile([C, HALF], FMM, tag="y_t")
            mrep_sb = big.tile([C, HALF], F32, tag="mrep_sb")
            w_t = big.tile([C, HALF], F32, tag="w_t")      # (1-m)/9
            xm = big.tile([C, HALF], F32, tag="xm")        # x*m
            ones_t = small.tile([C, C], FMM, tag="ones_t")
            ones_f = small.tile([C, C], F32, tag="ones_f")
            norm2 = small.tile([C, NLT], F32, tag="norm2")
            r2 = small.tile([C, NLT], F32, tag="r2")
            rs_a = small.tile([C, NLT], F32, tag="rs_a")
            rs_b = small.tile([C, NLT], F32, tag="rs_b")
            nt_a = small.tile([C, NLT], F32, tag="nt_a")
            eps_r = small.tile([C, NLT], F32, tag="eps_r")
            eps_c = small.tile([C, 1], F32, tag="eps_c")
            shift_c = small.tile([C, 1], F32, tag="shift_c")

            # ---- input DMAs: issue the critical ones from different idle
            # engines so the ~0.7us per-issue costs overlap
            Q = L // 4
            nc.sync.dma_start(xyh_sb[:, 0:YW // 2], xyh_d[:, 0:YW // 2])
            nc.sync.dma_start(xyh_sb[:, YW // 2:YW], xyh_d[:, YW // 2:YW])
            nc.scalar.dma_start(xt_sb[:, 0:Q // 2], xt_d[:, 0:Q // 2])
            nc.scalar.dma_start(xt_sb[:, Q // 2:Q], xt_d[:, Q // 2:Q])
            nc.sync.dma_start(x_sb[:, 0:Q], x_d[:, 0:Q])
            for g in range(1, 4):
                s = g * Q
                nc.sync.dma_start(xt_sb[:, s:s + Q], xt_d[:, s:s + Q])
            for g in range(1, 4):
                s = g * Q
                nc.sync.dma_start(x_sb[:, s:s + Q], x_d[:, s:s + Q])
            nc.sync.dma_start(mrep_sb[:], mrep_d[:])

            # ---- prologue ----
            nc.vector.memset(ones_f[:], 1.0)
            nc.vector.tensor_copy(ones_t[:], ones_f[:])
            nc.vector.memset(eps_c[:], EPS)
            nc.vector.memset(shift_c[:], -SHIFT)
            # pay the exp table-set load (~2.7us) during the DMA window;
            # exp is the only ACT table set this kernel uses
            warm2 = small.tile([C, 1], F32, tag="warm2")
            nc.scalar.activation(warm2[:], eps_c[:], AF.Exp)

            # rscale chain, pipelined in chunks of 8 l-tiles:
            # norm2 = sum_c (xt+eps)^2 (ACT Square + free-dim accumulate),
            # rscale = rsqrt(norm2) (DVE recip + ACT Sqrt seed + 2 Newton)
            I32 = mybir.dt.int32

            def rscale_chunk(l0, l1):
                cl = slice(l0, l1)
                for lt in range(l0, l1):
                    scr = sqs.tile([C, C], F32, tag="sq_scratch")
                    nc.scalar.activation(
                        scr[:], xt_sb[:, lt * C:(lt + 1) * C], AF.Square,
                        bias=eps_c[:], accum_out=norm2[:, lt:lt + 1],
                    )
                # rsqrt via bit-trick seed (no ACT table set needed) + 3 Newton
                nc.vector.tensor_scalar(nt_a[:, cl].bitcast(I32),
                                        norm2[:, cl].bitcast(I32), 1, None,
                                        op0=ALU.logical_shift_right)
                nc.vector.tensor_scalar(rs_b[:, cl].bitcast(I32),
                                        nt_a[:, cl].bitcast(I32),
                                        -1, 0x5f3759df,
                                        op0=ALU.mult, op1=ALU.add)
                src = rs_b
                dst = rs_a
                for _ in range(3):
                    nc.vector.tensor_mul(nt_a[:, cl], src[:, cl], src[:, cl])
                    nc.vector.tensor_mul(nt_a[:, cl], nt_a[:, cl], norm2[:, cl])
                    nc.vector.tensor_scalar(nt_a[:, cl], nt_a[:, cl], -0.5, 1.5,
                                            op0=ALU.mult, op1=ALU.add)
                    nc.vector.tensor_mul(dst[:, cl], src[:, cl], nt_a[:, cl])
                    src, dst = dst, src
                # 3 iterations (odd count) end with the result in rs_a
                nc.vector.tensor_scalar_mul(eps_r[:, cl], rs_a[:, cl], EPS)

            def kn_chunk(l0, l1):
                for lt in range(l0, l1):
                    nc.vector.tensor_scalar(
                        kn[:, lt * C:(lt + 1) * C], xt_sb[:, lt * C:(lt + 1) * C],
                        rs_a[:, lt:lt + 1], eps_r[:, lt:lt + 1],
                        op0=ALU.mult, op1=ALU.add,
                    )

            # y = 3x3 box filter (row filter on xyh -> y1, then col filter),
            # split by image-row ranges so block 0 is ready early
            xv = xyh_sb[:].rearrange("p (r j) -> p r j", j=64)
            yv = y1[:].rearrange("p (r j) -> p r j", j=64)

            # --- critical-path-ordered prologue emission ---
            # part A of y1: rows 0..18 (flat [0:1216)) - ready as soon as xyh
            nc.vector.tensor_add(y1[:, 1:1216], xyh_sb[:, 0:1215],
                                 xyh_sb[:, 1:1216])
            nc.vector.tensor_add(y1[:, 1:1216], y1[:, 1:1216],
                                 xyh_sb[:, 2:1217])
            nc.vector.tensor_add(yv[:, 0:19, 0:1], xv[:, 0:19, 0:1],
                                 xv[:, 0:19, 1:2])
            nc.vector.tensor_add(yv[:, 0:19, 63:64], xv[:, 0:19, 62:63],
                                 xv[:, 0:19, 63:64])
            # rscale chunk 0 (gates the first exps)
            rscale_chunk(0, 8)
            # y_t block 0 (gates the first mm1)
            nc.vector.tensor_add(y_t[:, 0:BLK], y1[:, 0:BLK],
                                 y1[:, 64:64 + BLK])
            nc.vector.tensor_add(y_t[:, 0:BLK], y_t[:, 0:BLK],
                                 y1[:, 128:128 + BLK])
            # xeps chunk 0 (gates mm1)
            nc.vector.tensor_scalar_add(xeps[:, 0:Q], x_sb[:, 0:Q], EPS)
            kn_chunk(0, 8)
            # part B of y1: rows 19..33 -> y_t block 1
            nc.vector.tensor_add(y1[:, 1216:YW - 1], xyh_sb[:, 1215:YW - 2],
                                 xyh_sb[:, 1216:YW - 1])
            nc.vector.tensor_add(y1[:, 1216:YW - 1], y1[:, 1216:YW - 1],
                                 xyh_sb[:, 1217:YW])
            nc.vector.tensor_add(yv[:, 19:34, 0:1], xv[:, 19:34, 0:1],
                                 xv[:, 19:34, 1:2])
            nc.vector.tensor_add(yv[:, 19:34, 63:64], xv[:, 19:34, 62:63],
                                 xv[:, 19:34, 63:64])
            nc.vector.tensor_add(y_t[:, BLK:HALF], y1[:, BLK:BLK + BLK],
                                 y1[:, BLK + 64:BLK + 64 + BLK])
            nc.vector.tensor_add(y_t[:, BLK:HALF], y_t[:, BLK:HALF],
                                 y1[:, BLK + 128:BLK + 128 + BLK])
            for g in range(1, 4):
                s = g * Q
                nc.vector.tensor_scalar_add(xeps[:, s:s + Q],
                                            x_sb[:, s:s + Q], EPS)
            rscale_chunk(8, NLT)

            # blend prep
            nc.vector.tensor_scalar(w_t[:], mrep_sb[:], -1.0 / 9.0, 1.0 / 9.0,
                                    op0=ALU.mult, op1=ALU.add)
            nc.vector.tensor_mul(xm[:], xyh_sb[:, 64:64 + HALF], mrep_sb[:])

            # ---- main loop ----
            for blk in range(HALF // BLK):
                rec = ps_rec.tile([C, BLK], F32, tag="rec")
                sums = ps_sum.tile([C, BLK], F32, tag="sums")
                u_prev = None
                v_prev = None
                w_queue = []   # (tile, idx) pending ones-mm, emitted lagged
                w_idx = 0

                def emit_ones(w, j):
                    for h2 in range(BLK // 512):
                        nc.tensor.matmul(
                            sums[:, h2 * 512:(h2 + 1) * 512],
                            mc(ones_t[:]),
                            mc(w[:, h2 * 512:(h2 + 1) * 512]),
                            start=(j == 0), stop=(j == NLT // 4 - 1),
                        )

                for lt in range(NLT):
                    # feed later kn chunks into the DVE stream mid-loop so
                    # they don't clog it ahead of the sum-tree adds
                    if blk == 0 and lt == 4:
                        kn_chunk(8, 16)
                    elif blk == 0 and lt == 10:
                        kn_chunk(16, 24)
                    elif blk == 0 and lt == 16:
                        kn_chunk(24, 32)
                    sc = ps_sc.tile([C, BLK], F32, tag="sc")
                    for h2 in range(BLK // 512):
                        cs = blk * BLK + h2 * 512
                        nc.tensor.matmul(
                            sc[:, h2 * 512:(h2 + 1) * 512],
                            mc(xeps[:, lt * C:(lt + 1) * C]),
                            mc(y_t[:, cs:cs + 512]),
                            start=True, stop=True,
                        )
                    u = upool.tile([C, BLK], FMM, tag="u")
                    nc.scalar.activation(u[:], sc[:], AF.Exp,
                                         bias=shift_c[:],
                                         scale=rs_a[:, lt:lt + 1])
                    for h2 in range(BLK // 512):
                        nc.tensor.matmul(
                            rec[:, h2 * 512:(h2 + 1) * 512],
                            mc(kn[:, lt * C:(lt + 1) * C]),
                            mc(u[:, h2 * 512:(h2 + 1) * 512]),
                            start=(lt == 0), stop=(lt == NLT - 1),
                        )
                    # column sums. Most l-tiles: 2-level pairwise tree on DVE,
                    # PE finishes with a lagged ones-mm per group of 4. The
                    # last 4 l-tiles: direct PE ones-mm (shortens the tail
                    # dependency chain exp->v->w->ones before the epilogue).
                    n_groups = NLT // 4 - 1 + 4  # 7 tree groups + 4 direct
                    if lt >= NLT - 4:
                        for h2 in range(BLK // 512):
                            nc.tensor.matmul(
                                sums[:, h2 * 512:(h2 + 1) * 512],
                                mc(ones_t[:]),
                                mc(u[:, h2 * 512:(h2 + 1) * 512]),
                                start=(w_idx == 0), stop=(w_idx == n_groups - 1),
                            )
                        w_idx += 1
                    elif lt % 2 == 0:
                        u_prev = u
                    else:
                        v = vpool.tile([C, BLK], F32, tag="v")
                        nc.vector.tensor_add(v[:], u_prev[:], u[:])
                        if v_prev is None:
                            v_prev = v
                        else:
                            w = wpool.tile([C, BLK], FMM, tag="w")
                            nc.vector.tensor_add(w[:], v_prev[:], v[:])
                            v_prev = None
                            w_queue.append((w, w_idx))
                            w_idx += 1
                            # lag the PE ones-mm 2 groups behind the DVE chain
                            if len(w_queue) > 2:
                                emit_ones(*w_queue.pop(0))
                    if lt == NLT - 5:
                        for w, j in w_queue:
                            emit_ones(w, j)
                        w_queue = []
                # epilogue: out = rec/sums * (1-m)/9 + x*m  (per-512 pipelined)
                for h2 in range(BLK // 512):
                    cs = blk * BLK + h2 * 512
                    sl = slice(h2 * 512, (h2 + 1) * 512)
                    R = opool.tile([C, 512], F32, tag="R")
                    nc.vector.reciprocal_approx_fast(R[:], sums[:, sl])
                    Rm = opool.tile([C, 512], F32, tag="Rm")
                    nc.vector.tensor_mul(Rm[:], R[:], w_t[:, cs:cs + 512])
                    ob = opool.tile([C, 512], F32, tag="ob")
                    nc.vector.tensor_mul(ob[:], rec[:, sl], Rm[:])
                    nc.vector.tensor_add(ob[:], ob[:], xm[:, cs:cs + 512])
                    nc.sync.dma_start(out_d[:, cs:cs + 512], ob[:])

    nc.finalize()
    return nc


def _get_program():
    if "nc" not in _CACHE:
        _CACHE["nc"] = _build_program()
    return _CACHE["nc"]


def _make_in_maps(fg, mk):
    in_maps = []
    for core in range(8):
        b, h = core // 2, core % 2
        x = np.ascontiguousarray(fg[b].reshape(C, L))
        # pre-tiled transpose: xt[p, t*128+c] = x[c, t*128+p]
        xt = np.ascontiguousarray(
            x.reshape(C, L // C, C).transpose(2, 1, 0).reshape(C, L))
        xi = fg[b].reshape(C, 64, 64)
        rows = np.zeros((C, 34, 64), np.float32)
        r0 = 32 * h - 1
        lo, hi = max(0, r0), min(64, r0 + 34)
        rows[:, lo - r0:hi - r0, :] = xi[:, lo:hi, :]
        xyh = np.ascontiguousarray(rows.reshape(C, YW))
        mrow = mk[b].reshape(1, L)[:, h * HALF:(h + 1) * HALF]
        mrep = np.ascontiguousarray(np.broadcast_to(mrow, (C, HALF)))
        in_maps.append({"x": x, "xt": xt, "xyh": xyh, "mrep": mrep})
    return in_maps


def kernel(foreground, mask):
    fg = np.ascontiguousarray(np.asarray(foreground, dtype=np.float32))
    mk = np.ascontiguousarray(np.asarray(mask, dtype=np.float32))
    nc = _get_program()
    in_maps = _make_in_maps(fg, mk)

    from concourse.bass_utils import run_bass_kernel_spmd
    res = run_bass_kernel_spmd(nc, in_maps, core_ids=list(range(8)))

    out = np.empty((4, C, L), np.float32)
    for core in range(8):
        b, h = core // 2, core % 2
        out[b][:, h * HALF:(h + 1) * HALF] = res.results[core]["out"]
    return out.reshape(4, C, 64, 64)


# revision 68
# speedup vs baseline: 1.0236x; 1.0236x over previous
"""Trainium2 Bass kernel for CAttention (contextual attention).

Math (per batch element, derived from the reference):
    x:    (c=128, h=64, w=64), flat (128, 4096); m: (1, 4096)
    k    = normalize_rows(x.reshape(c, hw).T + eps)          # (4096, 128)
    y    = 3x3 zero-padded box filter of x                   # (128, 4096)
      (the 3x3 sum-pool of the padded score map commutes with the 1x1-conv
       matmul and collapses onto x; border columns of the padded map only
       influence cropped-away outputs)
    S    = k @ y                                             # (4096 l, 4096 ij)
    att  = softmax over l (per column); constant-shift trick: softmax needs no
           per-column max because S is bounded (|S| <= ~34, col max >= ~11):
           u = exp(S - 20), att = u / colsum(u)
    rec  = k.T @ att                                         # (128, 4096)
    out  = rec * (1-m)/9 + x*m

Sharding: pure data parallel over batch (4) x output-column halves (2) = 8
cores, zero cross-core communication. Each core: full l = 4096, its 2048
output columns.

Per-core layout (l-orientation: l on partitions for scores):
    mm1:  scores[l_tile, ij] = xeps[:, l_tile].T @ y[:, ij]       (PE)
    exp:  u = Exp(scores * rscale[l] - 20)  per-partition scale    (ACT)
          (folds the k-row normalization into the softmax argument)
    mm2:  rec += kn[l_tile].T @ u   accumulated over 32 l-tiles    (PE)
    sums: sums += ones.T @ u        column sums, broadcast to 128p (PE)
    then R = 1/sums, out = rec*R*(1-m)/9 + x*m                     (DVE)
"""

import numpy as np

EPS = 1e-7
SHIFT = 20.0
C = 128          # channels
L = 4096         # spatial locations (l axis)
HALF = 2048      # output columns per core
BLK = 1024       # ij block (psum-bank sized: 2 banks)
NLT = 32         # l tiles of 128
YW = 2176        # xyh width: 34 padded image rows x 64
USE_F32R = True  # fast fp32 matmul mode on the PE (matmul operands stored as
                 # float32r; producers round to it, ~17-bit mantissa)

_CACHE = {}


def _build_program():
    import concourse.bass as bass
    import concourse.bacc as bacc
    import concourse.tile as tile
    import concourse.mybir as mybir

    F32 = mybir.dt.float32
    F32R = mybir.dt.float32r
    AF = mybir.ActivationFunctionType
    ALU = mybir.AluOpType

    nc = bacc.Bacc("TRN2", target_bir_lowering=False, num_swdge_queues=4)

    x_d = nc.dram_tensor("x", [C, L], F32, kind="ExternalInput")
    # xt pre-tiled on host to SBUF layout: xtt[p, t*128+c] = x[c, t*128+p]
    xt_d = nc.dram_tensor("xt", [C, L], F32, kind="ExternalInput")
    xyh_d = nc.dram_tensor("xyh", [C, YW], F32, kind="ExternalInput")
    mrep_d = nc.dram_tensor("mrep", [C, HALF], F32, kind="ExternalInput")
    out_d = nc.dram_tensor("out", [C, HALF], F32, kind="ExternalOutput")

    FMM = F32R if USE_F32R else F32  # dtype for matmul operand tiles

    def mc(ap):
        return ap

    with tile.TileContext(nc) as tc:
        with (
            tc.tile_pool(name="big", bufs=1) as big,
            tc.tile_pool(name="small", bufs=1) as small,
            tc.tile_pool(name="sqs", bufs=2) as sqs,
            tc.tile_pool(name="upool", bufs=6) as upool,
            tc.tile_pool(name="vpool", bufs=3) as vpool,
            tc.tile_pool(name="wpool", bufs=4) as wpool,
            tc.tile_pool(name="opool", bufs=3) as opool,
            tc.tile_pool(name="ps_sc", bufs=2, space=bass.MemorySpace.PSUM) as ps_sc,
            tc.tile_pool(name="ps_rec", bufs=1, space=bass.MemorySpace.PSUM) as ps_rec,
            tc.tile_pool(name="ps_sum", bufs=1, space=bass.MemorySpace.PSUM) as ps_sum,
        ):
            # ---- persistent SBUF tensors ----
            x_sb = big.tile([C, L], F32, tag="x_sb")
            xeps = big.tile([C, L], FMM, tag="xeps")
            xt_sb = big.tile([C, L], F32, tag="xt_sb")     # 32 tiles (128l, 128c)
            kn = big.tile([C, L], FMM, tag="kn")           # normalized k, l-major tiles
            xyh_sb = big.tile([C, YW], F32, tag="xyh_sb")
            y1 = big.tile([C, YW], F32, tag="y1")
            y_t = big.tile([C, HALF], FMM, tag="y_t")
            mrep_sb = big.tile([C, HALF], F32, tag="mrep_sb")
            w_t = big.tile([C, HALF], F32, tag="w_t")      # (1-m)/9
            xm = big.tile([C, HALF], F32, tag="xm")        # x*m
            ones_t = small.tile([C, C], FMM, tag="ones_t")
            ones_f = small.tile([C, C], F32, tag="ones_f")
            norm2 = small.tile([C, NLT], F32, tag="norm2")
            r2 = small.tile([C, NLT], F32, tag="r2")
            rs_a = small.tile([C, NLT], F32, tag="rs_a")
            rs_b = small.tile([C, NLT], F32, tag="rs_b")
            nt_a = small.tile([C, NLT], F32, tag="nt_a")
            eps_r = small.tile([C, NLT], F32, tag="eps_r")
            eps_c = small.tile([C, 1], F32, tag="eps_c")
            shift_c = small.tile([C, 1], F32, tag="shift_c")

            # ---- input DMAs: issue the critical ones from different idle
            # engines so the ~0.7us per-issue costs overlap
            Q = L // 4
            nc.sync.dma_start(xyh_sb[:], xyh_d[:])
            nc.scalar.dma_start(xt_sb[:, 0:Q], xt_d[:, 0:Q])
            nc.sync.dma_start(x_sb[:, 0:Q], x_d[:, 0:Q])
            for g in range(1, 4):
                s = g * Q
                nc.sync.dma_start(xt_sb[:, s:s + Q], xt_d[:, s:s + Q])
            for g in range(1, 4):
                s = g * Q
                nc.sync.dma_start(x_sb[:, s:s + Q], x_d[:, s:s + Q])
            nc.sync.dma_start(mrep_sb[:], mrep_d[:])

            # ---- prologue ----
            nc.vector.memset(ones_f[:], 1.0)
            nc.vector.tensor_copy(ones_t[:], ones_f[:])
            nc.vector.memset(eps_c[:], EPS)
            nc.vector.memset(shift_c[:], -SHIFT)
            # pay the exp table-set load (~2.7us) during the DMA window;
            # exp is the only ACT table set this kernel uses
            warm2 = small.tile([C, 1], F32, tag="warm2")
            nc.scalar.activation(warm2[:], eps_c[:], AF.Exp)

            # rscale chain, pipelined in chunks of 8 l-tiles:
            # norm2 = sum_c (xt+eps)^2 (ACT Square + free-dim accumulate),
            # rscale = rsqrt(norm2) (DVE recip + ACT Sqrt seed + 2 Newton)
            I32 = mybir.dt.int32

            def rscale_chunk(l0, l1):
                cl = slice(l0, l1)
                for lt in range(l0, l1):
                    scr = sqs.tile([C, C], F32, tag="sq_scratch")
                    nc.scalar.activation(
                        scr[:], xt_sb[:, lt * C:(lt + 1) * C], AF.Square,
                        bias=eps_c[:], accum_out=norm2[:, lt:lt + 1],
                    )
                # rsqrt via bit-trick seed (no ACT table set needed) + 3 Newton
                nc.vector.tensor_scalar(nt_a[:, cl].bitcast(I32),
                                        norm2[:, cl].bitcast(I32), 1, None,
                                        op0=ALU.logical_shift_right)
                nc.vector.tensor_scalar(rs_b[:, cl].bitcast(I32),
                                        nt_a[:, cl].bitcast(I32),
                                        -1, 0x5f3759df,
                                        op0=ALU.mult, op1=ALU.add)
                src = rs_b
                dst = rs_a
                for _ in range(3):
                    nc.vector.tensor_mul(nt_a[:, cl], src[:, cl], src[:, cl])
                    nc.vector.tensor_mul(nt_a[:, cl], nt_a[:, cl], norm2[:, cl])
                    nc.vector.tensor_scalar(nt_a[:, cl], nt_a[:, cl], -0.5, 1.5,
                                            op0=ALU.mult, op1=ALU.add)
                    nc.vector.tensor_mul(dst[:, cl], src[:, cl], nt_a[:, cl])
                    src, dst = dst, src
                # 3 iterations (odd count) end with the result in rs_a
                nc.vector.tensor_scalar_mul(eps_r[:, cl], rs_a[:, cl], EPS)

            def kn_chunk(l0, l1):
                for lt in range(l0, l1):
                    nc.vector.tensor_scalar(
                        kn[:, lt * C:(lt + 1) * C], xt_sb[:, lt * C:(lt + 1) * C],
                        rs_a[:, lt:lt + 1], eps_r[:, lt:lt + 1],
                        op0=ALU.mult, op1=ALU.add,
                    )

            # y = 3x3 box filter (row filter on xyh -> y1, then col filter),
            # split by image-row ranges so block 0 is ready early
            xv = xyh_sb[:].rearrange("p (r j) -> p r j", j=64)
            yv = y1[:].rearrange("p (r j) -> p r j", j=64)

            # --- critical-path-ordered prologue emission ---
            # part A of y1: rows 0..18 (flat [0:1216)) - ready as soon as xyh
            nc.vector.tensor_add(y1[:, 1:1216], xyh_sb[:, 0:1215],
                                 xyh_sb[:, 1:1216])
            nc.vector.tensor_add(y1[:, 1:1216], y1[:, 1:1216],
                                 xyh_sb[:, 2:1217])
            nc.vector.tensor_add(yv[:, 0:19, 0:1], xv[:, 0:19, 0:1],
                                 xv[:, 0:19, 1:2])
            nc.vector.tensor_add(yv[:, 0:19, 63:64], xv[:, 0:19, 62:63],
                                 xv[:, 0:19, 63:64])
            # rscale chunk 0 (gates the first exps)
            rscale_chunk(0, 8)
            # y_t block 0 (gates the first mm1)
            nc.vector.tensor_add(y_t[:, 0:BLK], y1[:, 0:BLK],
                                 y1[:, 64:64 + BLK])
            nc.vector.tensor_add(y_t[:, 0:BLK], y_t[:, 0:BLK],
                                 y1[:, 128:128 + BLK])
            # xeps chunk 0 (gates mm1)
            nc.vector.tensor_scalar_add(xeps[:, 0:Q], x_sb[:, 0:Q], EPS)
            kn_chunk(0, 8)
            # part B of y1: rows 19..33 -> y_t block 1
            nc.vector.tensor_add(y1[:, 1216:YW - 1], xyh_sb[:, 1215:YW - 2],
                                 xyh_sb[:, 1216:YW - 1])
            nc.vector.tensor_add(y1[:, 1216:YW - 1], y1[:, 1216:YW - 1],
                                 xyh_sb[:, 1217:YW])
            nc.vector.tensor_add(yv[:, 19:34, 0:1], xv[:, 19:34, 0:1],
                                 xv[:, 19:34, 1:2])
            nc.vector.tensor_add(yv[:, 19:34, 63:64], xv[:, 19:34, 62:63],
                                 xv[:, 19:34, 63:64])
            nc.vector.tensor_add(y_t[:, BLK:HALF], y1[:, BLK:BLK + BLK],
                                 y1[:, BLK + 64:BLK + 64 + BLK])
            nc.vector.tensor_add(y_t[:, BLK:HALF], y_t[:, BLK:HALF],
                                 y1[:, BLK + 128:BLK + 128 + BLK])
            for g in range(1, 4):
                s = g * Q
                nc.vector.tensor_scalar_add(xeps[:, s:s + Q],
                                            x_sb[:, s:s + Q], EPS)
            rscale_chunk(8, NLT)

            # blend prep
            nc.vector.tensor_scalar(w_t[:], mrep_sb[:], -1.0 / 9.0, 1.0 / 9.0,
                                    op0=ALU.mult, op1=ALU.add)
            nc.vector.tensor_mul(xm[:], xyh_sb[:, 64:64 + HALF], mrep_sb[:])

            # ---- main loop ----
            for blk in range(HALF // BLK):
                rec = ps_rec.tile([C, BLK], F32, tag="rec")
                sums = ps_sum.tile([C, BLK], F32, tag="sums")
                u_prev = None
                v_prev = None
                w_queue = []   # (tile, idx) pending ones-mm, emitted lagged
                w_idx = 0

                def emit_ones(w, j):
                    for h2 in range(BLK // 512):
                        nc.tensor.matmul(
                            sums[:, h2 * 512:(h2 + 1) * 512],
                            mc(ones_t[:]),
                            mc(w[:, h2 * 512:(h2 + 1) * 512]),
                            start=(j == 0), stop=(j == NLT // 4 - 1),
                        )

                for lt in range(NLT):
                    # feed later kn chunks into the DVE stream mid-loop so
                    # they don't clog it ahead of the sum-tree adds
                    if blk == 0 and lt == 4:
                        kn_chunk(8, 16)
                    elif blk == 0 and lt == 10:
                        kn_chunk(16, 24)
                    elif blk == 0 and lt == 16:
                        kn_chunk(24, 32)
                    sc = ps_sc.tile([C, BLK], F32, tag="sc")
                    for h2 in range(BLK // 512):
                        cs = blk * BLK + h2 * 512
                        nc.tensor.matmul(
                            sc[:, h2 * 512:(h2 + 1) * 512],
                            mc(xeps[:, lt * C:(lt + 1) * C]),
                            mc(y_t[:, cs:cs + 512]),
                            start=True, stop=True,
                        )
                    u = upool.tile([C, BLK], FMM, tag="u")
                    nc.scalar.activation(u[:], sc[:], AF.Exp,
                                         bias=shift_c[:],
                                         scale=rs_a[:, lt:lt + 1])
                    for h2 in range(BLK // 512):
                        nc.tensor.matmul(
                            rec[:, h2 * 512:(h2 + 1) * 512],
                            mc(kn[:, lt * C:(lt + 1) * C]),
                            mc(u[:, h2 * 512:(h2 + 1) * 512]),
                            start=(lt == 0), stop=(lt == NLT - 1),
                        )
                    # column sums. Most l-tiles: 2-level pairwise tree on DVE,
                    # PE finishes with a lagged ones-mm per group of 4. The
                    # last 4 l-tiles: direct PE ones-mm (shortens the tail
                    # dependency chain exp->v->w->ones before the epilogue).
                    n_groups = NLT // 4 - 1 + 4  # 7 tree groups + 4 direct
                    if lt >= NLT - 4:
                        for h2 in range(BLK // 512):
                            nc.tensor.matmul(
                                sums[:, h2 * 512:(h2 + 1) * 512],
                                mc(ones_t[:]),
                                mc(u[:, h2 * 512:(h2 + 1) * 512]),
                                start=(w_idx == 0), stop=(w_idx == n_groups - 1),
                            )
                        w_idx += 1
                    elif lt % 2 == 0:
                        u_prev = u
                    else:
                        v = vpool.tile([C, BLK], F32, tag="v")
                        nc.vector.tensor_add(v[:], u_prev[:], u[:])
                        if v_prev is None:
                            v_prev = v
                        else:
                            w = wpool.tile([C, BLK], FMM, tag="w")
                            nc.vector.tensor_add(w[:], v_prev[:], v[:])
                            v_prev = None
                            w_queue.append((w, w_idx))
                            w_idx += 1
                            # lag the PE ones-mm 2 groups behind the DVE chain
                            if len(w_queue) > 2:
                                emit_ones(*w_queue.pop(0))
                    if lt == NLT - 5:
                        for w, j in w_queue:
                            emit_ones(w, j)
                        w_queue = []
                # epilogue: out = rec/sums * (1-m)/9 + x*m  (per-512 pipelined)
                for h2 in range(BLK // 512):
                    cs = blk * BLK + h2 * 512
                    sl = slice(h2 * 512, (h2 + 1) * 512)
                    R = opool.tile([C, 512], F32, tag="R")
                    nc.vector.reciprocal_approx_fast(R[:], sums[:, sl])
                    Rm = opool.tile([C, 512], F32, tag="Rm")
                    nc.vector.tensor_mul(Rm[:], R[:], w_t[:, cs:cs + 512])
                    ob = opool.tile([C, 512], F32, tag="ob")
                    nc.vector.tensor_mul(ob[:], rec[:, sl], Rm[:])
                    nc.vector.tensor_add(ob[:], ob[:], xm[:, cs:cs + 512])
                    nc.sync.dma_start(out_d[:, cs:cs + 512], ob[:])

    nc.finalize()
    return nc


def _get_program():
    if "nc" not in _CACHE:
        _CACHE["nc"] = _build_program()
    return _CACHE["nc"]


def _make_in_maps(fg, mk):
    in_maps = []
    for core in range(8):
        b, h = core // 2, core % 2
        x = np.ascontiguousarray(fg[b].reshape(C, L))
        # pre-tiled transpose: xt[p, t*128+c] = x[c, t*128+p]
        xt = np.ascontiguousarray(
            x.reshape(C, L // C, C).transpose(2, 1, 0).reshape(C, L))
        xi = fg[b].reshape(C, 64, 64)
        rows = np.zeros((C, 34, 64), np.float32)
        r0 = 32 * h - 1
        lo, hi = max(0, r0), min(64, r0 + 34)
        rows[:, lo - r0:hi - r0, :] = xi[:, lo:hi, :]
        xyh = np.ascontiguousarray(rows.reshape(C, YW))
        mrow = mk[b].reshape(1, L)[:, h * HALF:(h + 1) * HALF]
        mrep = np.ascontiguousarray(np.broadcast_to(mrow, (C, HALF)))
        in_maps.append({"x": x, "xt": xt, "xyh": xyh, "mrep": mrep})
    return in_maps


def kernel(foreground, mask):
    fg = np.ascontiguousarray(np.asarray(foreground, dtype=np.float32))
    mk = np.ascontiguousarray(np.asarray(mask, dtype=np.float32))
    nc = _get_program()
    in_maps = _make_in_maps(fg, mk)

    from concourse.bass_utils import run_bass_kernel_spmd
    res = run_bass_kernel_spmd(nc, in_maps, core_ids=list(range(8)))

    out = np.empty((4, C, L), np.float32)
    for core in range(8):
        b, h = core // 2, core % 2
        out[b][:, h * HALF:(h + 1) * HALF] = res.results[core]["out"]
    return out.reshape(4, C, 64, 64)
